# revision 7
# baseline (speedup 1.0000x reference)
"""FrameDockingScoreModel forward kernel on 8 Trainium2 NeuronCores (Bass/Tile).

Contract: kernel(**inputs) takes FULL unsharded inputs (as from
setup_inputs()) and returns (s_rot, s_tr, t_rot, t_tr), float32 (B, 3) each.

Strategy (pure data parallel over B, per spec sharding hint):
  - Host folds the two e3nn tensor-product stages into ONE 126->12 linear map
    (W_mm) over per-edge nonlinear features, plus a small lane-side vector
    assembly for the 1e output (V2).
  - Device (per core, per chunk of edges in [128, C]-column layout):
      DVE/ACT: gaussian embeddings via A*rho^g, unit vectors, dots/crosses,
      feature outer products (126 feature planes).
      PE: per 512-edge block, 4 transposes lane->K-layout, one 126x12 matmul;
      coefficient planes transposed back to lane layout.
      DVE/ACT: V2 assembly + output packing; DMA out.
"""

import sys

sys.path.insert(0, "/opt/trn_rl_repo")

import numpy as np

P = 128
NG = 4
MAX_OFF = 5.0
DELTA = MAX_OFF / (NG - 1)
COEFF = -0.5 / DELTA ** 2
NU = -2.0 * COEFF * DELTA
CG = np.exp(COEFF * (np.arange(NG) * DELTA) ** 2)

C1_000 = (1.0 / 17.0) ** 0.5
C1_110 = (1.0 / 17.0) ** 0.5 / 3.0 ** 0.5
C1_011 = (3.0 / 9.0) ** 0.5 / 3.0 ** 0.5
C1_101 = C1_011
C1_111 = (3.0 / 9.0) ** 0.5 / 6.0 ** 0.5
C2_000 = (1.0 / 85.0) ** 0.5
C2_110 = (1.0 / 85.0) ** 0.5 / 3.0 ** 0.5
C2_011 = (3.0 / 45.0) ** 0.5 / 3.0 ** 0.5
C2_101 = C2_011
C2_111 = (3.0 / 45.0) ** 0.5 / 6.0 ** 0.5

N_CORES = 8
# Per-core column plan: chunks of C columns (C*128 edges each).
CHUNK_COLS = [144] * 6 + [116]          # 980 cols -> 125440 edges per core
E_PAD = P * sum(CHUNK_COLS)             # 125440
SG_MAX = 3                              # blocks (of 512 edges) per supergroup


def _precompute_w(w1_000, w1_110, w1_011, w1_101, w1_111,
                  w2_000, w2_110, w2_011, w2_101, w2_111):
    f64 = np.float64
    a = [np.asarray(w, f64) for w in (w1_000, w1_110, w1_011, w1_101, w1_111,
                                      w2_000, w2_110, w2_011, w2_101, w2_111)]
    (w1_000, w1_110, w1_011, w1_101, w1_111,
     w2_000, w2_110, w2_011, w2_101, w2_111) = a

    K3 = C2_000 * C1_000 * np.einsum('iju,uvw->ijvw', w1_000, w2_000)
    Kd = C2_000 * C1_110 * np.einsum('u,uvw->vw', w1_110, w2_000)
    Kg1 = C2_110 * C1_011 * np.einsum('iu,uw->iw', w1_011, w2_110)
    Kg2 = C2_110 * C1_101 * np.einsum('ju,uw->jw', w1_101, w2_110)
    Kc = C2_110 * C1_111 * np.einsum('u,uw->w', w1_111, w2_110)
    Qa = C2_011 * C1_000 * np.einsum('u,iju->ij', w2_011, w1_000)
    qd = C2_011 * C1_110 * float(w2_011 @ w1_110)
    B1 = C2_101 * C1_011 * np.einsum('iu,uv->iv', w1_011, w2_101)
    B2 = C2_101 * C1_101 * np.einsum('ju,uv->jv', w1_101, w2_101)
    b3v = C2_101 * C1_111 * np.einsum('u,uv->v', w1_111, w2_101)
    g1v = C2_111 * C1_011 * np.einsum('iu,u->i', w1_011, w2_111)
    g2v = C2_111 * C1_101 * np.einsum('ju,u->j', w1_101, w2_111)
    gamma3 = C2_111 * C1_111 * float(w2_111 @ w1_111)

    W = np.zeros((126, 12), dtype=f64)
    cg = CG
    W[0:64, 0:9] = (K3 * cg[:, None, None, None] * cg[None, :, None, None]
                    * cg[None, None, :, None]).reshape(64, 9)
    W[64:80, 9] = (Qa * cg[:, None] * cg[None, :]).reshape(16)
    W[80:96, 10] = (B1 * cg[:, None] * cg[None, :]).reshape(16)
    W[96:112, 11] = (B2 * cg[:, None] * cg[None, :]).reshape(16)
    W[112:116, 0:9] = Kg1 * cg[:, None]
    W[116:120, 0:9] = Kg2 * cg[:, None]
    W[120:124, 0:9] = Kd * cg[:, None]
    W[124, 0:9] = Kc
    W[125, 9] = qd

    consts = np.zeros((P, 16), dtype=np.float32)
    consts[:, 0:4] = (g1v * cg).astype(np.float32)[None, :]
    consts[:, 4:8] = (g2v * cg).astype(np.float32)[None, :]
    consts[:, 8:12] = (b3v * cg).astype(np.float32)[None, :]
    return W.astype(np.float32), consts, np.float32(gamma3)


def _build(chunk_cols, gamma3, debug=False):
    import concourse.bacc as bacc
    import concourse.mybir as mybir
    from concourse.tile import TileContext

    f32 = mybir.dt.float32
    ALU = mybir.AluOpType
    AF = mybir.ActivationFunctionType
    AX = mybir.AxisListType

    e_pad = P * sum(chunk_cols)
    nc = bacc.Bacc("TRN2", target_bir_lowering=False)

    lig_d = nc.declare_dram_parameter("lig", [e_pad, 12], f32, isOutput=False)
    rec_d = nc.declare_dram_parameter("rec", [e_pad, 12], f32, isOutput=False)
    w_d = nc.declare_dram_parameter("wmm", [126, 12], f32, isOutput=False)
    cg_d = nc.declare_dram_parameter("cgt", [P, 16], f32, isOutput=False)
    id_d = nc.declare_dram_parameter("ident", [P, P], f32, isOutput=False)
    out_d = nc.declare_dram_parameter("out", [e_pad, 12], f32, isOutput=True)
    dbg = {}
    if debug:
        c0 = chunk_cols[0]
        dbg['feat'] = nc.declare_dram_parameter("dbg_feat", [P, 126 * c0], f32, isOutput=True)
        dbg['coef'] = nc.declare_dram_parameter("dbg_coef", [P, 12 * c0], f32, isOutput=True)

    with TileContext(nc) as tc:
        with (
            tc.tile_pool(name="const", bufs=1) as pc,
            tc.tile_pool(name="pin", bufs=2) as pin,
            tc.tile_pool(name="prim", bufs=1) as pp,
            tc.tile_pool(name="feat", bufs=1) as pf,
            tc.tile_pool(name="ft", bufs=3) as pft,
            tc.tile_pool(name="ec", bufs=2) as pec,
            tc.tile_pool(name="coeft", bufs=1) as pct,
            tc.tile_pool(name="v2", bufs=1) as pv,
            tc.tile_pool(name="pout", bufs=2) as po,
            tc.tile_pool(name="psT", bufs=2, space="PSUM") as psT,
            tc.tile_pool(name="psC", bufs=2, space="PSUM") as psC,
            tc.tile_pool(name="psB", bufs=2, space="PSUM") as psB,
        ):
            wt = pc.tile([126, 12], f32)
            nc.sync.dma_start(out=wt[:], in_=w_d[:])
            cgt = pc.tile([P, 16], f32)
            nc.sync.dma_start(out=cgt[:], in_=cg_d[:])
            ident = pc.tile([P, P], f32)
            nc.sync.dma_start(out=ident[:], in_=id_d[:])

            ebase = 0
            for ci, C in enumerate(chunk_cols):
                n_e = P * C
                lig_v = lig_d[ebase:ebase + n_e, :].rearrange("(p c) f -> p (c f)", p=P)
                rec_v = rec_d[ebase:ebase + n_e, :].rearrange("(p c) f -> p (c f)", p=P)
                out_v = out_d[ebase:ebase + n_e, :].rearrange("(p c) f -> p (c f)", p=P)

                tin_l = pin.tile([P, C * 12], f32, tag="tin_l")
                tin_r = pin.tile([P, C * 12], f32, tag="tin_r")
                nc.sync.dma_start(out=tin_l[:], in_=lig_v)
                nc.sync.dma_start(out=tin_r[:], in_=rec_v)

                # ---- primitives (lane layout) ----
                e9 = pp.tile([P, 3, 3, C], f32, tag="e9")          # (atom, k, C)
                inl = tin_l[:].rearrange("p (c f) -> p f c", f=12)[:, 3:12, :] \
                    .rearrange("p (a k) c -> p a k c", a=3)
                inr = tin_r[:].rearrange("p (c f) -> p f c", f=12)[:, 3:12, :] \
                    .rearrange("p (a k) c -> p a k c", a=3)
                nc.vector.tensor_tensor(out=e9[:], in0=inl, in1=inr, op=ALU.subtract)

                sq9 = pp.tile([P, 3, 3, C], f32, tag="sq9")
                nc.scalar.activation(out=sq9[:], in_=e9[:], func=AF.Square)
                r2 = pp.tile([P, 3, C], f32, tag="r2")
                nc.vector.tensor_reduce(out=r2[:], in_=sq9[:].rearrange("p a k c -> p a c k"),
                                        axis=AX.X, op=ALU.add)
                lnr = pp.tile([P, 3, C], f32, tag="lnr")
                nc.scalar.activation(out=lnr[:], in_=r2[:], func=AF.Ln)
                rsq = pp.tile([P, 3, C], f32, tag="rsq")
                nc.scalar.activation(out=rsq[:], in_=lnr[:], func=AF.Exp, scale=-0.5)
                dd = pp.tile([P, 3, C], f32, tag="dd")
                nc.vector.tensor_tensor(out=dd[:], in0=r2[:], in1=rsq[:], op=ALU.mult)

                udup = pp.tile([P, 3, 6, C], f32, tag="udup")
                nc.vector.tensor_tensor(
                    out=udup[:, :, 0:3, :], in0=e9[:],
                    in1=rsq[:].unsqueeze(2).broadcast_to((P, 3, 3, C)), op=ALU.mult)
                nc.scalar.copy(out=udup[:, :, 3:6, :], in_=udup[:, :, 0:3, :])

                svec = pp.tile([P, 3, 4, C], f32, tag="svec")
                nc.scalar.activation(out=svec[:, :, 0, :], in_=r2[:], func=AF.Exp,
                                     scale=float(COEFF))
                rhop = pp.tile([P, 3, 3, C], f32, tag="rhop")
                nc.scalar.activation(out=rhop[:, :, 0, :], in_=dd[:], func=AF.Exp,
                                     scale=float(NU))
                nc.vector.tensor_tensor(out=rhop[:, :, 1, :], in0=rhop[:, :, 0, :],
                                        in1=rhop[:, :, 0, :], op=ALU.mult)
                nc.vector.tensor_tensor(out=rhop[:, :, 2, :], in0=rhop[:, :, 1, :],
                                        in1=rhop[:, :, 0, :], op=ALU.mult)
                nc.vector.tensor_tensor(
                    out=svec[:, :, 1:4, :],
                    in0=svec[:, :, 0, :].unsqueeze(2).broadcast_to((P, 3, 3, C)),
                    in1=rhop[:], op=ALU.mult)

                # ---- geometry ----
                pd = pp.tile([P, 3, 3, C], f32, tag="pd")
                nc.vector.tensor_tensor(
                    out=pd[:, 0:2, :, :],
                    in0=udup[:, 0, 0:3, :].unsqueeze(1).broadcast_to((P, 2, 3, C)),
                    in1=udup[:, 1:3, 0:3, :], op=ALU.mult)
                nc.vector.tensor_tensor(out=pd[:, 2, :, :], in0=udup[:, 1, 0:3, :],
                                        in1=udup[:, 2, 0:3, :], op=ALU.mult)
                dots = pp.tile([P, 3, C], f32, tag="dots")  # d12, d13, d23
                nc.vector.tensor_reduce(out=dots[:], in_=pd[:].rearrange("p q k c -> p q c k"),
                                        axis=AX.X, op=ALU.add)

                cr12 = pp.tile([P, 3, C], f32, tag="cr12")
                m1 = pp.tile([P, 3, C], f32, tag="m1")
                nc.vector.tensor_tensor(out=m1[:], in0=udup[:, 0, 1:4, :],
                                        in1=udup[:, 1, 2:5, :], op=ALU.mult)
                nc.vector.tensor_tensor(out=cr12[:], in0=udup[:, 0, 2:5, :],
                                        in1=udup[:, 1, 1:4, :], op=ALU.mult)
                nc.vector.tensor_tensor(out=cr12[:], in0=m1[:], in1=cr12[:], op=ALU.subtract)

                # ---- feature tile ----
                feat = pf.tile([P, 126, C], f32, tag="feat")
                # dc3 -> row 124
                tmp3 = pp.tile([P, 3, C], f32, tag="tmp3")
                nc.vector.tensor_tensor(out=tmp3[:], in0=cr12[:], in1=udup[:, 2, 0:3, :],
                                        op=ALU.mult)
                nc.vector.tensor_reduce(out=feat[:, 124, :],
                                        in_=tmp3[:].rearrange("p k c -> p c k"),
                                        axis=AX.X, op=ALU.add)
                nc.scalar.copy(out=feat[:, 125, :], in_=dots[:, 0, :])
                # f12 rows 64:80
                nc.vector.tensor_tensor(
                    out=feat[:, 64:80, :].rearrange("p (i j) c -> p i j c", i=4),
                    in0=svec[:, 0, :, :].unsqueeze(2).broadcast_to((P, 4, 4, C)),
                    in1=svec[:, 1, :, :].unsqueeze(1).broadcast_to((P, 4, 4, C)),
                    op=ALU.mult)
                # f13 rows 80:96
                nc.vector.tensor_tensor(
                    out=feat[:, 80:96, :].rearrange("p (i v) c -> p i v c", i=4),
                    in0=svec[:, 0, :, :].unsqueeze(2).broadcast_to((P, 4, 4, C)),
                    in1=svec[:, 2, :, :].unsqueeze(1).broadcast_to((P, 4, 4, C)),
                    op=ALU.mult)
                # f23 rows 96:112
                nc.vector.tensor_tensor(
                    out=feat[:, 96:112, :].rearrange("p (j v) c -> p j v c", j=4),
                    in0=svec[:, 1, :, :].unsqueeze(2).broadcast_to((P, 4, 4, C)),
                    in1=svec[:, 2, :, :].unsqueeze(1).broadcast_to((P, 4, 4, C)),
                    op=ALU.mult)
                # m123 rows 0:64 = f12 x s3
                nc.vector.tensor_tensor(
                    out=feat[:, 0:64, :].rearrange("p (ij v) c -> p ij v c", v=4),
                    in0=feat[:, 64:80, :].unsqueeze(2).broadcast_to((P, 16, 4, C)),
                    in1=svec[:, 2, :, :].unsqueeze(1).broadcast_to((P, 16, 4, C)),
                    op=ALU.mult)
                # g1 112:116, g2 116:120, ds3 120:124
                nc.vector.tensor_tensor(
                    out=feat[:, 112:116, :], in0=svec[:, 0, :, :],
                    in1=dots[:, 2, :].unsqueeze(1).broadcast_to((P, 4, C)), op=ALU.mult)
                nc.vector.tensor_tensor(
                    out=feat[:, 116:120, :], in0=svec[:, 1, :, :],
                    in1=dots[:, 1, :].unsqueeze(1).broadcast_to((P, 4, C)), op=ALU.mult)
                nc.vector.tensor_tensor(
                    out=feat[:, 120:124, :], in0=svec[:, 2, :, :],
                    in1=dots[:, 0, :].unsqueeze(1).broadcast_to((P, 4, C)), op=ALU.mult)

                if debug and ci == 0:
                    nc.sync.dma_start(out=dbg['feat'][:],
                                      in_=feat[:].rearrange("p f c -> p (f c)"))

                # ---- PE: transposes + matmul + transpose-back ----
                nblk = C // 4
                coefT = pct.tile([P, C * 12], f32, tag="coefT")
                b0 = 0
                while b0 < nblk:
                    nb = min(SG_MAX, nblk - b0)
                    coefP = psC.tile([P, 512], f32, tag="coefP")
                    for g in range(nb):
                        b = b0 + g
                        pst = psT.tile([126, 512], f32, tag="psT")
                        for k in range(4):
                            nc.tensor.transpose(out=pst[:, k * 128:(k + 1) * 128],
                                                in_=feat[:, :, b * 4 + k],
                                                identity=ident[:])
                        ftile = pft.tile([126, 512], f32, tag="ft")
                        nc.scalar.copy(out=ftile[0:64, :], in_=pst[0:64, :])
                        nc.vector.tensor_copy(ftile[64:126, :], pst[64:126, :])
                        nc.tensor.matmul(out=coefP[g * 32:g * 32 + 12, :],
                                         lhsT=wt[:], rhs=ftile[:],
                                         start=True, stop=True)
                    ecC = pec.tile([P, 512], f32, tag="ecC")
                    nc.scalar.copy(out=ecC[:], in_=coefP[:])
                    pb = psB.tile([P, 512], f32, tag="psB")
                    for k in range(4):
                        nc.tensor.transpose(out=pb[:, k * 128:(k + 1) * 128],
                                            in_=ecC[:, k * 128:(k + 1) * 128],
                                            identity=ident[:])
                    # evac3 -> coefT[p, c*12 + j], c = (b0+g)*4 + k
                    dst = coefT[:, b0 * 48: b0 * 48 + nb * 48] \
                        .rearrange("p (g k f) -> p k g f", k=4, f=12)
                    src = pb[:].rearrange("p (k g r) -> p k g r", k=4, g=4)[:, :, 0:nb, 0:12]
                    nc.vector.tensor_copy(dst, src)
                    b0 += nb

                if debug and ci == 0:
                    nc.sync.dma_start(out=dbg['coef'][:], in_=coefT[:])

                # ---- gamma/beta lane contractions ----
                g4 = pp.tile([P, 3, 4, C], f32, tag="g4")
                nc.vector.tensor_tensor(
                    out=g4[:], in0=svec[:],
                    in1=cgt[:, 0:12].rearrange("p (t g) -> p t g", t=3)
                        .unsqueeze(3).broadcast_to((P, 3, 4, C)),
                    op=ALU.mult)
                gam = pp.tile([P, 3, C], f32, tag="gam")  # gamma1, gamma2, beta3
                nc.vector.tensor_reduce(out=gam[:], in_=g4[:].rearrange("p t g c -> p t c g"),
                                        axis=AX.X, op=ALU.add)

                # ---- V2 assembly ----
                ctf = coefT[:].rearrange("p (c f) -> p f c", f=12)
                v2a = pv.tile([P, 3, C], f32, tag="v2a")
                t1 = pv.tile([P, 3, C], f32, tag="t1")
                u1 = udup[:, 0, 0:3, :]
                u2 = udup[:, 1, 0:3, :]
                u3 = udup[:, 2, 0:3, :]
                nc.vector.tensor_tensor(
                    out=v2a[:], in0=ctf[:, 9, :].unsqueeze(1).broadcast_to((P, 3, C)),
                    in1=u3, op=ALU.mult)
                nc.vector.tensor_tensor(
                    out=t1[:], in0=ctf[:, 10, :].unsqueeze(1).broadcast_to((P, 3, C)),
                    in1=u2, op=ALU.mult)
                nc.vector.tensor_tensor(out=v2a[:], in0=v2a[:], in1=t1[:], op=ALU.add)
                nc.vector.tensor_tensor(
                    out=t1[:], in0=ctf[:, 11, :].unsqueeze(1).broadcast_to((P, 3, C)),
                    in1=u1, op=ALU.mult)
                nc.vector.tensor_tensor(out=v2a[:], in0=v2a[:], in1=t1[:], op=ALU.add)
                nc.vector.tensor_tensor(
                    out=t1[:], in0=gam[:, 2, :].unsqueeze(1).broadcast_to((P, 3, C)),
                    in1=cr12[:], op=ALU.mult)
                nc.vector.tensor_tensor(out=v2a[:], in0=v2a[:], in1=t1[:], op=ALU.add)
                # w vector (dup'd)
                wd = pp.tile([P, 6, C], f32, tag="wd")
                nc.vector.tensor_tensor(
                    out=wd[:, 0:3, :], in0=gam[:, 0, :].unsqueeze(1).broadcast_to((P, 3, C)),
                    in1=u2, op=ALU.mult)
                nc.vector.tensor_tensor(
                    out=t1[:], in0=gam[:, 1, :].unsqueeze(1).broadcast_to((P, 3, C)),
                    in1=u1, op=ALU.mult)
                nc.vector.tensor_tensor(out=wd[:, 0:3, :], in0=wd[:, 0:3, :], in1=t1[:],
                                        op=ALU.add)
                nc.vector.scalar_tensor_tensor(out=wd[:, 0:3, :], in0=cr12[:],
                                               scalar=float(gamma3), in1=wd[:, 0:3, :],
                                               op0=ALU.mult, op1=ALU.add)
                nc.scalar.copy(out=wd[:, 3:6, :], in_=wd[:, 0:3, :])
                # cx = w x u3
                cx = pv.tile([P, 3, C], f32, tag="cx")
                nc.vector.tensor_tensor(out=cx[:], in0=wd[:, 1:4, :],
                                        in1=udup[:, 2, 2:5, :], op=ALU.mult)
                nc.vector.tensor_tensor(out=t1[:], in0=wd[:, 2:5, :],
                                        in1=udup[:, 2, 1:4, :], op=ALU.mult)
                nc.vector.tensor_tensor(out=cx[:], in0=cx[:], in1=t1[:], op=ALU.subtract)

                outt = po.tile([P, C * 12], f32, tag="outt")
                ov = outt[:].rearrange("p (c f) -> p f c", f=12)
                nc.vector.tensor_tensor(out=ov[:, 9:12, :], in0=v2a[:], in1=cx[:], op=ALU.add)
                # S2 copy (split halves between engines)
                h = C // 2
                nc.scalar.copy(out=ov[:, 0:9, 0:h], in_=ctf[:, 0:9, 0:h])
                nc.vector.tensor_copy(ov[:, 0:9, h:C], ctf[:, 0:9, h:C])

                nc.sync.dma_start(out=out_v, in_=outt[:])
                ebase += n_e

    nc.finalize()
    return nc


_CACHE = {}


def _get_nc(chunk_cols, gamma3, debug=False):
    key = (tuple(chunk_cols), float(gamma3), debug)
    if key not in _CACHE:
        _CACHE[key] = _build(chunk_cols, gamma3, debug)
    return _CACHE[key]


def kernel(ligand, receptor, w1_000, w1_110, w1_011, w1_101, w1_111,
           w2_000, w2_110, w2_011, w2_101, w2_111):
    from concourse.bass_utils import run_bass_kernel_spmd

    f32 = np.float32
    lig = np.ascontiguousarray(np.asarray(ligand, f32).reshape(-1, 12))
    rec = np.ascontiguousarray(np.asarray(receptor, f32).reshape(-1, 12))
    B = lig.shape[0]
    W, consts, gamma3 = _precompute_w(w1_000, w1_110, w1_011, w1_101, w1_111,
                                      w2_000, w2_110, w2_011, w2_101, w2_111)
    ident = np.eye(P, dtype=f32)

    nc = _get_nc(CHUNK_COLS, gamma3)

    per = (B + N_CORES - 1) // N_CORES
    assert per <= E_PAD, (per, E_PAD)
    in_maps = []
    for c in range(N_CORES):
        s, e = c * per, min((c + 1) * per, B)
        lpad = np.zeros((E_PAD, 12), f32)
        rpad = np.zeros((E_PAD, 12), f32)
        lpad[0:e - s] = lig[s:e]
        rpad[0:e - s] = rec[s:e]
        in_maps.append({"lig": lpad, "rec": rpad, "wmm": W, "cgt": consts,
                        "ident": ident})

    res = run_bass_kernel_spmd(nc, in_maps, core_ids=list(range(N_CORES)))
    out = np.empty((B, 12), f32)
    for c in range(N_CORES):
        s, e = c * per, min((c + 1) * per, B)
        out[s:e] = res.results[c]["out"][0:e - s]
    return (out[:, 0:3].copy(), out[:, 3:6].copy(),
            out[:, 6:9].copy(), out[:, 9:12].copy())
